# revision 42
# baseline (speedup 1.0000x reference)
"""Self-contained Trainium2 Bass kernel for a 6-layer post-LN transformer
encoder (B=2, S=2048, D=1024, H=16, F=4096, V=32000).

Sharding: sequence-parallel across 8 NeuronCores. Core c handles batch
b = c // 4, sequence slice [lc*512, (lc+1)*512); per layer K/V are
AllGathered in bf16 within each batch's 4 cores.

v3 (this file): keeps v2's two-block software pipeline (attention of the
next token-half overlaps Wo/FFN/QKV of the current half via a micro-task
queue) and adds:
  - GS=2 attention super-groups: one ScalarE exp instruction covers two
    key chunks (2 PSUM banks, single accumulation group per bank via
    start-on-first / stop-on-last in disjoint half-bank regions), with a
    1-super-group software pipeline: scores/exp of group g+1 are emitted
    before AV of group g so the exp latency never head-of-line-blocks
    the in-order PE queue.
  - K/V residents split by token-half (kA/kB, vA/vB, bufs=1 each): the
    half-(l+1) fill's WAR dependency now expires a full block earlier,
    removing the layer-boundary serialization.
  - Z (softmax denominator) path with no DRAM bounces: AV row 64 is
    DVE-copied to a partition-0 staging tile; pass-2 evacuation does
    z = z1 + z2 and a per-pair reciprocal; attn_norm broadcasts 1/Z with
    one matmul per pair (lhsT = ones[1,64], rhs = zinv[1,2,HT]).
  - Micro-task queue with weight *prefetch*: each task splits into a
    DMA prefetch (issued 2 tasks ahead) and a compute body, so dense
    matmuls never wait on their weight DMA. W2 is fetched as one 1 MB
    tile per m-chunk. KV projection is split into 8 K + 4 V subtasks.
  - DMA queue split: weights on qSPDynamicHW (nc.sync), K/V staging
    stores and gather fills on qActDynamicHW (nc.scalar).
PSUM budget: sc 2x2 + av 2 + acc 2 (also hosts LN stats / broadcasts)
= 8 banks.
"""

import math
from contextlib import ExitStack

import ml_dtypes
import numpy as np

import concourse.bass as bass
import concourse.tile as tile
from concourse import bacc, mybir
from concourse.bass import IndirectOffsetOnAxis
from concourse.bass_utils import run_bass_kernel_spmd

dt = mybir.dt
Alu = mybir.AluOpType
Act = mybir.ActivationFunctionType

V, L, D, S, F, H, B = 32000, 6, 1024, 2048, 4096, 16, 2
DK = D // H          # 64
NC = 8               # cores
NT = (B * S) // NC   # 512 tokens per core
HT = NT // 2         # 256 tokens per half
CH = D // 128        # 8 feature chunks
FCH = F // 128       # 32 ffn chunks
EPS = 1e-5
SQRT_D = math.sqrt(D)
GROUPS = [[0, 1, 2, 3], [4, 5, 6, 7]]
NSG = 4              # super-groups per (pass, pair): 8 kc / 2
PF_AHEAD = 3         # weight-prefetch lookahead in the task queue
PASS1_TGT = 22       # sg index by which pre-barrier queue work must drain
RESERVE = 0.0        # queue cost carried into the next block's start


class C:
    """Pools, constants and dram handles threaded through emission."""
    pass


# ------------------------------------------------------------------
# small per-layer params
# ------------------------------------------------------------------

def load_smalls(nc, c, l):
    sm = {}
    for nm, dram, width in (
        ("bq", c.bq_d, CH), ("bk", c.bk_d, CH), ("bo", c.bo_d, CH),
        ("b2", c.b2_d, CH), ("b1", c.b1_d, FCH), ("g1", c.g1_d, CH),
        ("be1", c.be1_d, CH), ("g2", c.g2_d, CH), ("be2", c.be2_d, CH),
    ):
        t = c.small.tile([128, width], dt.float32, tag=nm, name=f"{nm}{l}")
        nc.sync.dma_start(out=t[:, :],
                          in_=dram[l].rearrange("(c p) -> p c", p=128))
        sm[nm] = t
    bv_l = c.bv_d[l]
    bv_bc = c.small.tile([128, D], dt.bfloat16, tag="bv", name=f"bv{l}")
    nc.gpsimd.dma_start(
        out=bv_bc[:, :],
        in_=bass.AP(tensor=bv_l.tensor, offset=bv_l.offset,
                    ap=[[0, 128]] + list(bv_l.ap)))
    sm["bv"] = bv_bc
    return sm


# ------------------------------------------------------------------
# projections + gathers (direct emission; queue wraps these per-chunk)
# ------------------------------------------------------------------

def k_chunk(nc, c, l, h, m, x_bf, sm, k_dr_v, wm):
    """K projection chunk m for token-half h; wm preloaded [128,CH,128]."""
    ts = slice(h * HT, (h + 1) * HT)
    ps = c.acc.tile([128, HT], dt.float32, tag="acc", name=f"psk{l}_{h}_{m}")
    for kc in range(CH):
        nc.tensor.matmul(ps[:, :], lhsT=wm[:, kc, :], rhs=x_bf[:, kc, ts],
                         start=(kc == 0), stop=(kc == CH - 1))
    ko = c.kown.tile([128, HT], dt.bfloat16, tag="ko", name=f"ko{l}_{h}_{m}")
    nc.vector.tensor_scalar(out=ko[:, :], in0=ps[:, :],
                            scalar1=sm["bk"][:, m:m + 1], scalar2=None,
                            op0=Alu.add)
    nc.scalar.dma_start(out=k_dr_v[:, m, :], in_=ko[:, :])


def v_chunk(nc, c, l, h, vf, x_bf, sm, v_dr_v, wvn):
    """V projection feature-slice vf (256 wide) for half h."""
    fs = slice(vf * 256, (vf + 1) * 256)
    for tc4 in range(2):
        tok = slice(h * HT + tc4 * 128, h * HT + (tc4 + 1) * 128)
        ps = c.acc.tile([128, HT], dt.float32, tag="acc",
                        name=f"psv{l}_{h}_{tc4}_{vf}")
        for kc in range(CH):
            nc.tensor.matmul(ps[:, :], lhsT=x_bf[:, kc, tok],
                             rhs=wvn[:, kc, :],
                             start=(kc == 0), stop=(kc == CH - 1))
        vo = c.kown.tile([128, HT], dt.bfloat16, tag="ko",
                         name=f"vo{l}_{h}_{tc4}_{vf}")
        nc.vector.tensor_tensor(out=vo[:, :], in0=ps[:, :],
                                in1=sm["bv"][:, fs], op=Alu.add)
        nc.scalar.dma_start(out=v_dr_v[:, tc4, fs], in_=vo[:, :])


def kv_dram_tiles(nc, c, l, h):
    k_dr = c.dram.tile([CH * 128 * HT], dt.bfloat16, tag=f"kod{h}",
                       name=f"kod{l}_{h}")
    k_dr_v = k_dr[:].rearrange("(ch p q) -> p ch q", p=128, q=HT)
    v_dr = c.dram.tile([2 * 128 * D], dt.bfloat16, tag=f"vod{h}",
                       name=f"vod{l}_{h}")
    v_dr_v = v_dr[:].rearrange("(t p f) -> p t f", p=128, f=D)
    return k_dr, k_dr_v, v_dr, v_dr_v


def kick_k_gather(nc, c, l, h, k_dr):
    kg = c.dram.tile([4, CH * 128 * HT], dt.bfloat16, tag=f"kgd{h}",
                     name=f"kgd{l}_{h}")
    nc.gpsimd.collective_compute("AllGather", Alu.bypass, replica_groups=GROUPS,
                                 ins=[k_dr[:].opt()], outs=[kg[:, :].opt()])
    return kg


def kick_v_gather(nc, c, l, h, v_dr):
    vg = c.dram.tile([4, 2 * 128 * D], dt.bfloat16, tag=f"vgd{h}",
                     name=f"vgd{l}_{h}")
    nc.gpsimd.collective_compute("AllGather", Alu.bypass, replica_groups=GROUPS,
                                 ins=[v_dr[:].opt()], outs=[vg[:, :].opt()])
    return vg


def q_proj_chunk(nc, c, l, h, m, x_bf, sm, q_bf, wm):
    ts = slice(h * HT, (h + 1) * HT)
    ps = c.acc.tile([128, HT], dt.float32, tag="acc", name=f"psq{l}_{h}_{m}")
    for kc in range(CH):
        nc.tensor.matmul(ps[:, :], lhsT=wm[:, kc, :], rhs=x_bf[:, kc, ts],
                         start=(kc == 0), stop=(kc == CH - 1))
    nc.vector.tensor_scalar(out=q_bf[:, m, ts], in0=ps[:, :],
                            scalar1=sm["bq"][:, m:m + 1], scalar2=None,
                            op0=Alu.add)


def load_w_chunk(nc, c, w_r, m, name):
    wm = c.wpool.tile([128, CH, 128], dt.bfloat16, tag="w", name=name)
    nc.sync.dma_start(out=wm[:, :, :], in_=w_r[:, :, m * 128:(m + 1) * 128])
    return wm


def kH_fill(nc, c, l, h, kg, kH):
    """Gathered K of half h -> resident kH [128, CH, 4*HT]."""
    kga = kg[:, :].rearrange("g (ch p q) -> p ch g q", ch=CH, p=128, q=HT)
    for g in range(4):
        nc.scalar.dma_start(out=kH[:, :, g * HT:(g + 1) * HT],
                            in_=kga[:, :, g, :])


def vH_fill(nc, c, l, h, vg, vH):
    """Gathered V of half h -> resident vH [128, 8, H, 65] (+ones col)."""
    vga = vg[:, :].rearrange("g (t p f) -> p g t f", p=128, f=D)
    ov = vH[:, :, :, 0:64].rearrange("p (g t) hh e -> p g t hh e", g=4)
    for g in range(4):
        for t in range(2):
            nc.scalar.dma_start(
                out=ov[:, g, t],
                in_=vga[:, g, t, :].rearrange("p (hh e) -> p hh e", e=64))


# ------------------------------------------------------------------
# layernorm pieces
# ------------------------------------------------------------------

def ln_stats(nc, c, x_in, name):
    """x_in [128, CH, HT] f32 -> st psum [1, 2, HT] = (sum, sumsq)."""
    xbs = c.hp.tile([128, CH, 2, HT], dt.bfloat16, tag="h", name=f"xbs_{name}")
    nc.vector.tensor_copy(out=xbs[:, :, 0, :], in_=x_in[:, :, :])
    nc.vector.tensor_tensor(out=xbs[:, :, 1, :], in0=xbs[:, :, 0, :],
                            in1=xbs[:, :, 0, :], op=Alu.mult)
    st = c.acc.tile([1, 2, HT], dt.float32, tag="acc", name=f"st_{name}")
    for kc in range(CH):
        nc.tensor.matmul(st[:, :, :], lhsT=c.ones_col[:, :],
                         rhs=xbs[:, kc, :, :], start=(kc == 0),
                         stop=(kc == CH - 1))
    return st


def ln_acts(nc, c, st, name):
    """rs = exp(-0.5*ln(var+eps)); returns rsmu bf16 [1,2,HT] (rs, mu*rs)."""
    sm_t = c.zp.tile([1, 2, HT], dt.float32, tag="sm", name=f"sm_{name}")
    nc.vector.tensor_scalar(out=sm_t[:, :, :], in0=st[:, :, :],
                            scalar1=1.0 / D, scalar2=None, op0=Alu.mult)
    var = c.zp.tile([1, HT], dt.float32, tag="var", name=f"var_{name}")
    nc.vector.tensor_tensor(out=var[:, :], in0=sm_t[:, 0, :], in1=sm_t[:, 0, :],
                            op=Alu.mult)
    nc.vector.tensor_tensor(out=var[:, :], in0=sm_t[:, 1, :], in1=var[:, :],
                            op=Alu.subtract)
    rsmu = c.zp.tile([1, 2, HT], dt.bfloat16, tag="rsmu", name=f"rsmu_{name}")
    nc.scalar.activation(out=var[:, :], in_=var[:, :], func=Act.Ln,
                         bias=c.eps_sb[:, :], scale=1.0)
    nc.scalar.activation(out=rsmu[:, 0, :], in_=var[:, :], func=Act.Exp,
                         bias=0.0, scale=-0.5)
    nc.vector.tensor_tensor(out=rsmu[:, 1, :], in0=sm_t[:, 0, :],
                            in1=rsmu[:, 0, :], op=Alu.mult)
    return rsmu


def ln_apply(nc, c, rsmu, x_in, g_sb, be_sb, x_out, xb_out, name):
    """x_out = (x_in*rs - mu*rs)*g + be; xb_out = bf16(x_out)."""
    bc = c.acc.tile([128, 2, HT], dt.float32, tag="acc", name=f"bc_{name}")
    nc.tensor.matmul(bc[:, :, :], lhsT=c.ones_row[:, :], rhs=rsmu[:, :, :],
                     start=True, stop=True)
    for ch in range(CH):
        nc.vector.tensor_tensor(out=x_out[:, ch, :], in0=x_in[:, ch, :],
                                in1=bc[:, 0, :], op=Alu.mult)
        nc.vector.tensor_tensor(out=x_out[:, ch, :], in0=x_out[:, ch, :],
                                in1=bc[:, 1, :], op=Alu.subtract)
        nc.vector.tensor_scalar(out=x_out[:, ch, :], in0=x_out[:, ch, :],
                                scalar1=g_sb[:, ch:ch + 1],
                                scalar2=be_sb[:, ch:ch + 1],
                                op0=Alu.mult, op1=Alu.add)
        nc.vector.tensor_copy(out=xb_out[:, ch, :], in_=x_out[:, ch, :])


# ------------------------------------------------------------------
# attention
# ------------------------------------------------------------------

class AttnState:
    def __init__(self, o_sb, zinv):
        self.o_sb = o_sb
        self.zinv = zinv


def attn_half(nc, c, l, h, q_bf, kh, vh, queue, name):
    """Scores/exp/AV for token-half h of layer l, 1-super-group pipelined,
    popping micro-tasks from `queue`. kh/vh = (kA, kB), (vA, vB) of layer
    l. Pass 1 = A keys, pass 2 = B keys (queue drained to the gather
    barrier first). Returns AttnState."""
    ts = slice(h * HT, (h + 1) * HT)
    # z1 holds pass-1 partial Z per head; pass-2 evac reads it and then
    # overwrites the same bytes with 1/Z (the partial is dead by then).
    z1 = c.z1p.tile([1, CH, 2, HT], dt.bfloat16, tag="z1", name=f"z1_{name}")
    zinv = z1
    o_sb = c.op.tile([128, CH, HT], dt.bfloat16, tag="osb", name=f"osb_{name}")

    pend = None  # (ps, pair, sg, at2, avs)
    hold_zs = [None]

    def do_av(item):
        ps, pair, sg, at2, avs = item
        for jj in range(2):
            ck = sg * 2 + jj
            for i in range(2):
                nc.tensor.matmul(avs[i][0:65, :],
                                 lhsT=vh[ps][:, ck, 2 * pair + i, :],
                                 rhs=at2[:, i, jj, :],
                                 start=(sg == 0 and jj == 0),
                                 stop=(sg == NSG - 1 and jj == 1),
                                 skip_group_check=True)

    def do_evac(item):
        ps, pair, sg, at2, avs = item
        for i in range(2):
            if ps == 0:
                nc.vector.tensor_copy(out=o_sb[64 * i:64 * i + 64, pair, :],
                                      in_=avs[i][0:64, :])
                nc.vector.tensor_copy(out=z1[0:1, pair, i, :],
                                      in_=avs[i][64:65, :])
            else:
                nc.vector.tensor_tensor(
                    out=o_sb[64 * i:64 * i + 64, pair, :],
                    in0=o_sb[64 * i:64 * i + 64, pair, :],
                    in1=avs[i][0:64, :], op=Alu.add)
                if pair % 2 == 0 and i == 0:
                    zs = c.zp.tile([1, 2, 2, HT], dt.float32, tag="zs",
                                   name=f"zs_{name}_{pair}")
                    hold_zs[0] = zs
                nc.vector.tensor_tensor(out=hold_zs[0][:, pair % 2, i, :],
                                        in0=z1[0:1, pair, i, :],
                                        in1=avs[i][64:65, :], op=Alu.add)
        if ps == 1 and pair % 2 == 1:
            # 1/Z = exp(-ln(Z)) on ScalarE, batched over two head-pairs
            # to halve ACT instruction overhead; overwrites the dead
            # partial-Z bytes in z1.
            nc.scalar.activation(out=hold_zs[0][:, :, :, :],
                                 in_=hold_zs[0][:, :, :, :],
                                 func=Act.Ln, bias=0.0, scale=1.0)
            nc.scalar.activation(out=zinv[0:1, pair - 1:pair + 1, :, :],
                                 in_=hold_zs[0][:, :, :, :],
                                 func=Act.Exp, bias=0.0, scale=-1.0)

    for ps in range(2):
        if ps == 1:
            queue.drain_to_barrier()
        for pair in range(CH):
            avs = [c.avp.tile([128, HT], dt.float32, tag="av",
                              name=f"av_{name}_{ps}_{pair}_{i}")
                   for i in range(2)]
            for sg in range(NSG):
                sc2 = c.scp.tile([128, 2, 2, HT], dt.float32, tag="sc",
                                 name=f"sc_{name}_{ps}_{pair}_{sg}")
                for jj in range(2):
                    ck = sg * 2 + jj
                    for i in range(2):
                        nc.tensor.matmul(
                            sc2[:, i, jj, :],
                            lhsT=kh[ps][64 * i:64 * i + 64, pair,
                                        ck * 128:(ck + 1) * 128],
                            rhs=q_bf[64 * i:64 * i + 64, pair, ts],
                            start=(jj == 0), stop=(jj == 1),
                            tile_position=(64 * i, 0),
                            skip_group_check=True)
                at2 = c.atp.tile([128, 2, 2, HT], dt.bfloat16, tag="at",
                                 name=f"at_{name}_{ps}_{pair}_{sg}")
                nc.scalar.activation(out=at2[:, :, :, :],
                                     in_=sc2[:, :, :, :],
                                     func=Act.Exp, bias=0.0, scale=0.125)
                if pend is not None:
                    do_av(pend)
                    if pend[2] == NSG - 1:
                        do_evac(pend)
                # adaptive pacing: finish pre-barrier work by sg TGT of
                # pass 1 so the LN2->KV->gather->fill chain for pass 2
                # executes with maximal lead; spread the rest evenly but
                # keep RESERVE units back — they carry into the next
                # block to cover its attn_norm head-of-line stall.
                idx = (ps * CH + pair) * NSG + sg
                pre, post = queue.split_cost()
                npop = 2 * CH * NSG
                budget = max(0.0, post - RESERVE) / max(1.0, npop - idx)
                if pre:
                    budget += pre / max(1.0, PASS1_TGT - idx)
                queue.pop(budget)
                pend = (ps, pair, sg, at2, avs)
    do_av(pend)
    do_evac(pend)
    return AttnState(o_sb, zinv)


def attn_norm(nc, c, astate, name):
    """softmax-normalize: o_sb *= broadcast(1/Z) (in place)."""
    for pair in range(CH):
        bz = c.acc.tile([128, 2, HT], dt.float32, tag="acc",
                        name=f"bz_{name}_{pair}")
        nc.tensor.matmul(bz[0:64, :, :], lhsT=c.ones_row[:, 0:64],
                         rhs=astate.zinv[0:1, pair, :, :],
                         start=True, stop=True)
        for i in range(2):
            nc.vector.tensor_tensor(
                out=astate.o_sb[64 * i:64 * i + 64, pair, :],
                in0=astate.o_sb[64 * i:64 * i + 64, pair, :],
                in1=bz[0:64, i, :], op=Alu.mult)
    return astate.o_sb


# ------------------------------------------------------------------
# micro-task queue with prefetch
# ------------------------------------------------------------------

class Task:
    __slots__ = ("cost", "fn", "pf", "pf_done", "chain")

    def __init__(self, cost, fn, pf=None, chain=False):
        self.cost = cost
        self.fn = fn
        self.pf = pf
        self.pf_done = pf is None
        self.chain = chain


class TaskQueue:
    """Ordered task list. pop(budget) emits compute bodies until the
    popped cost reaches `budget`, keeping the next PF_AHEAD tasks'
    prefetches (weight DMAs) issued ahead. A barrier marks the gather
    kick; pass 2 of attention drains to it first."""

    def __init__(self):
        self.tasks = []
        self.barrier_idx = None
        self.carry_idx = None

    def add(self, cost, fn, pf=None, chain=False):
        self.tasks.append(Task(cost, fn, pf, chain))

    def add_barrier(self):
        self.barrier_idx = len(self.tasks)

    def add_carry_mark(self):
        self.carry_idx = len(self.tasks)

    def _prefetch_ahead(self):
        for t in self.tasks[:PF_AHEAD + 1]:
            if not t.pf_done:
                t.pf()
                t.pf_done = True

    def _run_one(self):
        t = self.tasks.pop(0)
        if self.barrier_idx is not None:
            self.barrier_idx -= 1
        if self.carry_idx is not None:
            self.carry_idx -= 1
        if not t.pf_done:
            t.pf()
            t.pf_done = True
        t.fn()
        return t.cost

    def pop(self, budget):
        spent = 0.0
        while self.tasks and (spent < budget or self.tasks[0].chain):
            self._prefetch_ahead()
            spent += self._run_one()

    def split_cost(self):
        """(cost before barrier, cost at/after barrier)."""
        bi = self.barrier_idx if self.barrier_idx is not None else 0
        pre = sum(t.cost for t in self.tasks[:bi])
        post = sum(t.cost for t in self.tasks[bi:])
        return pre, post

    def drain_to_barrier(self):
        while self.tasks and self.barrier_idx is not None and self.barrier_idx > 0:
            self._prefetch_ahead()
            self._run_one()
        self.barrier_idx = None

    def drain_to_carry(self):
        """Emit everything except the carry-safe tail (or all, if no
        carry mark was set)."""
        if self.carry_idx is None:
            self.drain()
            return
        while self.tasks and self.carry_idx > 0:
            self._prefetch_ahead()
            self._run_one()
        self.carry_idx = None

    def drain(self):
        while self.tasks:
            self._prefetch_ahead()
            self._run_one()


# ------------------------------------------------------------------
# the dense task list for block(h, l)
# ------------------------------------------------------------------

def build_phi2(nc, c, l, h, layers, st):
    """Queue: Wo(h)+LN1 -> FFN(h)+LN2 -> KV/Q(h,l+1) + gather kicks +
    staging fills for (l+1) second half."""
    q = TaskQueue()
    ts = slice(h * HT, (h + 1) * HT)
    sm = st["sm"]
    x = st["x"]
    last_layer = (l == layers - 1)
    wo_r = c.wo_d[l].rearrange("(kc p) f -> p kc f", p=128)
    w1_r = c.w1_d[l].rearrange("(kc p) f -> p kc f", p=128)
    w2_r = c.w2_d[l].rearrange("(kc p) f -> p kc f", p=128)

    o_nbf = st["o_norm"]  # normalized attention output (set in emit_block)
    hold = {}

    # ---- Wo + residual into x (in place) ----
    def wo_pf(m):
        def pf():
            hold[f"wo{m}"] = load_w_chunk(nc, c, wo_r, m, f"wo{l}_{h}_{m}")
        return pf

    def wo_m(m):
        def fn():
            wm = hold.pop(f"wo{m}")
            ps = c.acc.tile([128, HT], dt.float32, tag="acc",
                            name=f"pso{l}_{h}_{m}")
            for kc in range(CH):
                nc.tensor.matmul(ps[:, :], lhsT=wm[:, kc, :],
                                 rhs=o_nbf[:, kc, :],
                                 start=(kc == 0), stop=(kc == CH - 1))
            nc.vector.tensor_scalar(out=ps[:, :], in0=ps[:, :],
                                    scalar1=sm["bo"][:, m:m + 1], scalar2=None,
                                    op0=Alu.add)
            nc.vector.tensor_tensor(out=x[:, m, ts], in0=x[:, m, ts],
                                    in1=ps[:, :], op=Alu.add)
        return fn
    for m in range(CH):
        q.add(1.1, wo_m(m), wo_pf(m))

    # ---- LN1 (in place: x half-slice becomes x1n) ----
    x1n = x[:, :, ts]
    x1nb = c.x1nbp.tile([128, CH, HT], dt.bfloat16, tag="x1nb",
                        name=f"x1nb{l}_{h}")

    def ln1_stats():
        hold["st1"] = ln_stats(nc, c, x[:, :, ts], f"l{l}h{h}a")

    def ln1_acts():
        hold["rsmu1"] = ln_acts(nc, c, hold.pop("st1"), f"l{l}h{h}a")

    def ln1_apply():
        ln_apply(nc, c, hold.pop("rsmu1"), x[:, :, ts], sm["g1"], sm["be1"],
                 x1n, x1nb, f"l{l}h{h}a")
    q.add(2.0, ln1_stats)
    q.add(0.3, ln1_acts)
    q.add(2.5, ln1_apply)

    # ---- FFN W1 (relu into h_bf) ----
    def w1_pf(m):
        def pf():
            hold[f"w1{m}"] = load_w_chunk(nc, c, w1_r, m, f"w1_{l}_{h}_{m}")
        return pf

    def w1_m(m):
        def fn():
            if m == 0:
                hold["h_bf"] = c.hp.tile([128, FCH, HT], dt.bfloat16,
                                         tag="h", name=f"h{l}_{h}")
            h_bf = hold["h_bf"]
            w1m = hold.pop(f"w1{m}")
            ps = c.acc.tile([128, HT], dt.float32, tag="acc",
                            name=f"ps1{l}_{h}_{m}")
            for kc in range(CH):
                nc.tensor.matmul(ps[:, :], lhsT=w1m[:, kc, :],
                                 rhs=x1nb[:, kc, :],
                                 start=(kc == 0), stop=(kc == CH - 1))
            nc.vector.tensor_scalar(out=h_bf[:, m, :], in0=ps[:, :],
                                    scalar1=sm["b1"][:, m:m + 1],
                                    scalar2=0.0, op0=Alu.add, op1=Alu.max)
        return fn
    for m in range(FCH):
        q.add(1.1, w1_m(m))
        q.tasks[-1].pf = w1_pf(m)
        q.tasks[-1].pf_done = False

    # ---- FFN W2 + residual into x1n (in place) ----
    def w2_pf(m):
        def pf():
            w2m = c.w2p.tile([128, FCH, 128], dt.bfloat16, tag="w2",
                             name=f"w2_{l}_{h}_{m}")
            nc.sync.dma_start(out=w2m[:, :, :],
                              in_=w2_r[:, :, m * 128:(m + 1) * 128])
            hold[f"w2{m}"] = w2m
        return pf

    def w2_m(m):
        def fn():
            h_bf = hold["h_bf"]
            w2m = hold.pop(f"w2{m}")
            ps = c.acc.tile([128, HT], dt.float32, tag="acc",
                            name=f"ps2{l}_{h}_{m}")
            for kc in range(FCH):
                nc.tensor.matmul(ps[:, :], lhsT=w2m[:, kc, :],
                                 rhs=h_bf[:, kc, :],
                                 start=(kc == 0), stop=(kc == FCH - 1))
            nc.vector.tensor_scalar(out=ps[:, :], in0=ps[:, :],
                                    scalar1=sm["b2"][:, m:m + 1], scalar2=None,
                                    op0=Alu.add)
            nc.vector.tensor_tensor(out=x1n[:, m, :], in0=x1n[:, m, :],
                                    in1=ps[:, :], op=Alu.add)
        return fn
    for m in range(CH):
        q.add(4.2, w2_m(m), w2_pf(m))

    # ---- LN2 -> x_next slices ----
    def ln2_stats():
        hold["st2"] = ln_stats(nc, c, x1n, f"l{l}h{h}b")

    def ln2_acts():
        hold["rsmu2"] = ln_acts(nc, c, hold.pop("st2"), f"l{l}h{h}b")

    xn_b, xbn_b = st["x_next"], st["xb_next"]

    def ln2_apply():
        ln_apply(nc, c, hold.pop("rsmu2"), x1n, sm["g2"], sm["be2"],
                 xn_b[:, :, ts], xbn_b[:, :, ts], f"l{l}h{h}b")
    q.add(2.0, ln2_stats)
    q.add(0.3, ln2_acts)
    q.add(2.5, ln2_apply)

    # ---- next layer KV/Q + gathers ----
    if not last_layer:
        sm_n = st["sm_next"]
        wk_r = c.wk_d[l + 1].rearrange("(kc p) f -> p kc f", p=128)
        wv_r = c.wv_d[l + 1].rearrange("(kc p) f -> p kc f", p=128)
        wq_r = c.wq_d[l + 1].rearrange("(kc p) f -> p kc f", p=128)

        def kv_alloc():
            k_dr, k_dr_v, v_dr, v_dr_v = kv_dram_tiles(nc, c, l + 1, h)
            hold["kdr"] = (k_dr, k_dr_v, v_dr, v_dr_v)
        q.add(0.05, kv_alloc)

        def k_pf(m):
            def pf():
                hold[f"wk{m}"] = load_w_chunk(nc, c, wk_r, m,
                                              f"wk{l + 1}_{h}_{m}")
            return pf

        def k_m(m):
            def fn():
                k_chunk(nc, c, l + 1, h, m, xbn_b, sm_n, hold["kdr"][1],
                        hold.pop(f"wk{m}"))
            return fn
        for m in range(CH):
            q.add(1.1, k_m(m), k_pf(m))

        def kick_k():
            st["kg"][h] = kick_k_gather(nc, c, l + 1, h, hold["kdr"][0])
        q.add(0.05, kick_k)

        def v_pf(vf):
            def pf():
                wvn = c.wvp.tile([128, CH, 256], dt.bfloat16, tag="wv",
                                 name=f"wv{l + 1}_{h}_{vf}")
                nc.sync.dma_start(out=wvn[:, :, :],
                                  in_=wv_r[:, :, vf * 256:(vf + 1) * 256])
                hold[f"wv{vf}"] = wvn
            return pf

        def v_m(vf):
            def fn():
                v_chunk(nc, c, l + 1, h, vf, xbn_b, sm_n, hold["kdr"][3],
                        hold.pop(f"wv{vf}"))
            return fn
        for vf in range(4):
            q.add(2.2, v_m(vf), v_pf(vf))

        def kick_v():
            st["vg"][h] = kick_v_gather(nc, c, l + 1, h, hold.pop("kdr")[2])
        q.add(0.05, kick_v)

        if h == 1:
            # attention(A, l+1) pass 2 (B keys) must follow this block's
            # gather kicks + resident fills: barrier covers the fills so
            # drain_to_barrier emits them before any pass-2 score.
            kB_b, vB_b = st["kB_next"], st["vB_next"]

            def fills():
                kH_fill(nc, c, l + 1, 1, st["kg"][1], kB_b)
                vH_fill(nc, c, l + 1, 1, st["vg"][1], vB_b)
            q.add(0.2, fills)
            q.add_barrier()

        # only the Q tail is carry-safe: it reads xb_next/q_next bound
        # here and nothing the next block's staging overwrites.
        q.add_carry_mark()
        q_next_b = st["q_next"]

        def q_pf(m):
            def pf():
                hold[f"wq{m}"] = load_w_chunk(nc, c, wq_r, m,
                                              f"wq{l + 1}_{h}_{m}")
            return pf

        def q_m(m):
            def fn():
                q_proj_chunk(nc, c, l + 1, h, m, xbn_b, sm_n,
                             q_next_b, hold.pop(f"wq{m}"))
            return fn
        for m in range(CH):
            q.add(1.1, q_m(m), q_pf(m))
    return q


# ------------------------------------------------------------------
# program
# ------------------------------------------------------------------

def build_program(layers=L):
    nc = bacc.Bacc("TRN2", target_bir_lowering=False, debug=False,
                   num_devices=NC)
    c = C()
    c.tok_d = nc.dram_tensor("tokens_c", [NT], dt.int32, kind="ExternalInput")
    c.emb_d = nc.dram_tensor("emb", [V, D], dt.float32, kind="ExternalInput")
    c.pe_d = nc.dram_tensor("pe_fm", [D, NT], dt.float32, kind="ExternalInput")
    for nm in ("Wq", "Wk", "Wv", "Wo"):
        setattr(c, nm.lower() + "_d",
                nc.dram_tensor(nm, [layers, D, D], dt.bfloat16,
                               kind="ExternalInput"))
    c.w1_d = nc.dram_tensor("W1", [layers, D, F], dt.bfloat16,
                            kind="ExternalInput")
    c.w2_d = nc.dram_tensor("W2", [layers, F, D], dt.bfloat16,
                            kind="ExternalInput")
    for nm, shp in (("bq", D), ("bk", D), ("bv", D), ("bo", D), ("b1", F),
                    ("b2", D), ("g1", D), ("be1", D), ("g2", D), ("be2", D)):
        setattr(c, nm + "_d",
                nc.dram_tensor(nm, [layers, shp], dt.float32,
                               kind="ExternalInput"))
    c.out_d = nc.dram_tensor("out_fm", [D, NT], dt.float32,
                             kind="ExternalOutput")

    with tile.TileContext(nc) as tc, ExitStack() as ctx:
        # PSUM: sc 2x2 + av 2 + acc 2 = 8 banks
        c.scp = ctx.enter_context(tc.tile_pool(name="scp", bufs=2, space="PSUM"))
        c.avp = ctx.enter_context(tc.tile_pool(name="avp", bufs=2, space="PSUM"))
        c.acc = ctx.enter_context(tc.tile_pool(name="accp", bufs=2, space="PSUM"))
        # SBUF
        c.consts = ctx.enter_context(tc.tile_pool(name="consts", bufs=1))
        c.small = ctx.enter_context(tc.tile_pool(name="small", bufs=2))
        c.xp = ctx.enter_context(tc.tile_pool(name="xp", bufs=2))
        c.xbp = ctx.enter_context(tc.tile_pool(name="xbp", bufs=1))
        c.x1nbp = ctx.enter_context(tc.tile_pool(name="x1nbp", bufs=1))
        c.kap = ctx.enter_context(tc.tile_pool(name="kap", bufs=1))
        c.kbp = ctx.enter_context(tc.tile_pool(name="kbp", bufs=1))
        c.vap = ctx.enter_context(tc.tile_pool(name="vap", bufs=1))
        c.vbp = ctx.enter_context(tc.tile_pool(name="vbp", bufs=1))
        c.kown = ctx.enter_context(tc.tile_pool(name="kown", bufs=2))
        c.qp = ctx.enter_context(tc.tile_pool(name="qp", bufs=2))
        c.op = ctx.enter_context(tc.tile_pool(name="op", bufs=2))
        c.hp = ctx.enter_context(tc.tile_pool(name="hp", bufs=1))
        c.atp = ctx.enter_context(tc.tile_pool(name="atp", bufs=2))
        c.zp = ctx.enter_context(tc.tile_pool(name="zp", bufs=1))
        c.z1p = ctx.enter_context(tc.tile_pool(name="z1p", bufs=1))
        c.wpool = ctx.enter_context(tc.tile_pool(name="wp", bufs=4))
        c.w2p = ctx.enter_context(tc.tile_pool(name="w2p", bufs=2))
        c.wvp = ctx.enter_context(tc.tile_pool(name="wvp", bufs=2))
        c.dram = ctx.enter_context(tc.tile_pool(name="dram", bufs=2,
                                                space="DRAM"))

        ident = c.consts.tile([128, 128], dt.float32, name="ident")
        from concourse.masks import make_identity
        make_identity(nc, ident[:, :])
        ones_col = c.consts.tile([128, 1], dt.bfloat16, name="ones_col")
        nc.vector.memset(ones_col[:, :], 1.0)
        ones_row = c.consts.tile([1, 128], dt.bfloat16, name="ones_row")
        nc.vector.memset(ones_row[:, :], 1.0)
        eps_sb = c.consts.tile([1, 1], dt.float32, name="eps_sb")
        nc.vector.memset(eps_sb[:, :], EPS)
        c.ones_col, c.ones_row, c.eps_sb = ones_col, ones_row, eps_sb

        # ---------------- embedding ----------------
        tok_sb = c.consts.tile([128, 4], dt.int32, name="tok_sb")
        nc.sync.dma_start(out=tok_sb[:, :],
                          in_=c.tok_d[:].rearrange("(c p) -> p c", p=128))
        pe_sb = c.xp.tile([128, CH, NT], dt.float32, tag="x", name="pe_sb")
        nc.sync.dma_start(out=pe_sb[:, :, :],
                          in_=c.pe_d[:, :].rearrange("(c p) q -> p c q", p=128))
        x = c.xp.tile([128, CH, NT], dt.float32, tag="x", name="x0")
        for tc4 in range(4):
            gath = c.wvp.tile([128, D], dt.float32, tag="wv", name=f"gath{tc4}")
            nc.gpsimd.indirect_dma_start(
                out=gath[:, :], out_offset=None, in_=c.emb_d[:, :],
                in_offset=IndirectOffsetOnAxis(ap=tok_sb[:, tc4:tc4 + 1],
                                               axis=0))
            for fc in range(CH):
                tp = c.acc.tile([128, HT], dt.float32, tag="acc",
                                name=f"tp{tc4}_{fc}")
                nc.tensor.transpose(tp[:, 0:128],
                                    gath[:, fc * 128:(fc + 1) * 128],
                                    ident[:, :])
                nc.vector.tensor_scalar(
                    out=x[:, fc, tc4 * 128:(tc4 + 1) * 128], in0=tp[:, 0:128],
                    scalar1=float(SQRT_D), scalar2=None, op0=Alu.mult)
        nc.vector.tensor_tensor(out=x[:, :, :], in0=x[:, :, :],
                                in1=pe_sb[:, :, :], op=Alu.add)
        x_bf = c.xbp.tile([128, CH, NT], dt.bfloat16, tag="xb", name="xb0")
        nc.vector.tensor_copy(out=x_bf[:, :, :], in_=x[:, :, :])

        # ---------------- prologue: layer-0 QKV + gathers ----------------
        sm0 = load_smalls(nc, c, 0)
        st = dict(sm=sm0, x=x, xb=x_bf, kg={}, vg={})
        wk_r0 = c.wk_d[0].rearrange("(kc p) f -> p kc f", p=128)
        wv_r0 = c.wv_d[0].rearrange("(kc p) f -> p kc f", p=128)
        wq_r0 = c.wq_d[0].rearrange("(kc p) f -> p kc f", p=128)
        # K projections + kicks for both halves first (gathers fly while
        # V/Q projections run), then V + kicks, then Q.
        kgs, vgs, drs = {}, {}, {}
        for h in (0, 1):
            drs[h] = kv_dram_tiles(nc, c, 0, h)
            for m in range(CH):
                wm = load_w_chunk(nc, c, wk_r0, m, f"wk0_{h}_{m}")
                k_chunk(nc, c, 0, h, m, x_bf, sm0, drs[h][1], wm)
            kgs[h] = kick_k_gather(nc, c, 0, h, drs[h][0])
        for h in (0, 1):
            for vf in range(4):
                wvn = c.wvp.tile([128, CH, 256], dt.bfloat16, tag="wv",
                                 name=f"wv0_{h}_{vf}")
                nc.sync.dma_start(
                    out=wvn[:, :, :],
                    in_=wv_r0[:, :, vf * 256:(vf + 1) * 256])
                v_chunk(nc, c, 0, h, vf, x_bf, sm0, drs[h][3], wvn)
            vgs[h] = kick_v_gather(nc, c, 0, h, drs[h][2])
        q_bf = c.qp.tile([128, CH, NT], dt.bfloat16, tag="q", name="q0")
        for h in (0, 1):
            for m in range(CH):
                wm = load_w_chunk(nc, c, wq_r0, m, f"wq0_{h}_{m}")
                q_proj_chunk(nc, c, 0, h, m, x_bf, sm0, q_bf, wm)
        st["q"] = q_bf

        def alloc_kv(tagA, l):
            kA = c.kap.tile([128, CH, 4 * HT], dt.bfloat16, tag="ka",
                            name=f"kA{l}") if tagA == "A" else \
                 c.kbp.tile([128, CH, 4 * HT], dt.bfloat16, tag="kb",
                            name=f"kB{l}")
            vA = c.vap.tile([128, CH, H, 65], dt.bfloat16, tag="va",
                            name=f"vA{l}") if tagA == "A" else \
                 c.vbp.tile([128, CH, H, 65], dt.bfloat16, tag="vb",
                            name=f"vB{l}")
            nc.vector.memset(vA[:, :, :, 64:65], 1.0)
            return kA, vA

        kA0, vA0 = alloc_kv("A", 0)
        kB0, vB0 = alloc_kv("B", 0)
        kH_fill(nc, c, 0, 0, kgs[0], kA0)
        vH_fill(nc, c, 0, 0, vgs[0], vA0)
        kH_fill(nc, c, 0, 1, kgs[1], kB0)
        vH_fill(nc, c, 0, 1, vgs[1], vB0)
        st["kh"], st["vh"] = (kA0, kB0), (vA0, vB0)
        # attention(A, 0) with an empty queue (prologue bubble, once)
        st["attn"] = attn_half(nc, c, 0, 0, q_bf, st["kh"], st["vh"],
                               TaskQueue(), "l0h0")

        # ---------------- blocks ----------------
        for l in range(layers):
            for h in (0, 1):
                emit_block(nc, c, l, h, layers, st, alloc_kv)

        nc.sync.dma_start(
            out=c.out_d[:, :].rearrange("(ch p) q -> p ch q", p=128),
            in_=st["x"][:, :, :])

    nc.compile()
    return nc


def emit_block(nc, c, l, h, layers, st, alloc_kv):
    last_layer = (l == layers - 1)
    # --- step 0: per-block staging ---
    if h == 0:
        st["x_next"] = c.xp.tile([128, CH, NT], dt.float32, tag="x",
                                 name=f"x{l + 1}")
        st["xb_next"] = c.xbp.tile([128, CH, NT], dt.bfloat16, tag="xb",
                                   name=f"xb{l + 1}")
        if not last_layer:
            st["sm_next"] = load_smalls(nc, c, l + 1)
            st["q_next"] = c.qp.tile([128, CH, NT], dt.bfloat16, tag="q",
                                     name=f"q{l + 1}")
    else:
        if not last_layer:
            # A-half residents for layer l+1 (gather landed in block(l,0));
            # the WAR on kA(l)/vA(l) expired after attn(B,l) pass 1.
            kAn, vAn = alloc_kv("A", l + 1)
            kH_fill(nc, c, l + 1, 0, st["kg"][0], kAn)
            vH_fill(nc, c, l + 1, 0, st["vg"][0], vAn)
            kBn, vBn = alloc_kv("B", l + 1)
            st["kA_next"], st["vA_next"] = kAn, vAn
            st["kB_next"], st["vB_next"] = kBn, vBn

    # --- step 1: normalize this half's attention output ---
    astate = st.pop("attn")
    st["o_norm"] = attn_norm(nc, c, astate, f"l{l}h{h}")

    # --- step 1.5: drain tasks carried from the previous block; they
    # depend only on older state, so they execute while the o_norm DVE
    # chain percolates, keeping the PE fed at the block boundary.
    carry = st.pop("carry", None)
    if carry is not None:
        carry.drain()

    # --- step 2: build dense task queue ---
    queue = build_phi2(nc, c, l, h, layers, st)

    # --- step 3: next-half attention interleaved with the queue ---
    nh, nl = (1, l) if h == 0 else (0, l + 1)
    if not (last_layer and h == 1):
        if h == 0:
            qn, khn, vhn = st["q"], st["kh"], st["vh"]
        else:
            qn = st["q_next"]
            khn = (st["kA_next"], st["kB_next"])
            vhn = (st["vA_next"], st["vB_next"])
        st["attn"] = attn_half(nc, c, nl, nh, qn, khn, vhn, queue,
                               f"l{nl}h{nh}")
        queue.drain_to_carry()
        st["carry"] = queue
    else:
        queue.drain()

    # --- step 4: state rotation at end of layer ---
    if h == 1:
        st["x"], st["xb"] = st["x_next"], st["xb_next"]
        if not last_layer:
            st["q"] = st["q_next"]
            st["sm"] = st["sm_next"]
            st["kh"] = (st["kA_next"], st["kB_next"])
            st["vh"] = (st["vA_next"], st["vB_next"])


# ------------------------------------------------------------------
# host side
# ------------------------------------------------------------------

_PROG = {}


def _get_prog(layers=L):
    if layers not in _PROG:
        _PROG[layers] = build_program(layers)
    return _PROG[layers]


def _host_inputs(inputs, layers=L):
    bf16 = ml_dtypes.bfloat16
    f32 = np.float32
    tokens = np.asarray(inputs["tokens"])
    pos = np.arange(S)[:, None].astype(f32)
    freq = np.exp(np.arange(0, D, 2).astype(f32) * -(math.log(10000.0) / D))
    pe = np.zeros((S, D), dtype=f32)
    pe[:, 0::2] = np.sin(pos * freq)
    pe[:, 1::2] = np.cos(pos * freq)
    shared = {
        "emb": np.ascontiguousarray(np.asarray(inputs["emb"], dtype=f32)),
    }
    for nm in ("Wq", "Wk", "Wv", "Wo", "W1", "W2"):
        shared[nm] = np.ascontiguousarray(
            np.asarray(inputs[nm])[:layers].astype(bf16))
    for nm in ("bq", "bk", "bv", "bo", "b1", "b2", "g1", "be1", "g2", "be2"):
        shared[nm] = np.ascontiguousarray(
            np.asarray(inputs[nm])[:layers].astype(f32))

    in_maps = []
    for core in range(NC):
        b, lc = core // 4, core % 4
        m = dict(shared)
        m["tokens_c"] = np.ascontiguousarray(tokens[b, lc * NT:(lc + 1) * NT])
        m["pe_fm"] = np.ascontiguousarray(pe[lc * NT:(lc + 1) * NT, :].T)
        in_maps.append(m)
    return in_maps


def run(inputs, layers=L, trace=False):
    nc = _get_prog(layers)
    in_maps = _host_inputs(inputs, layers)
    for attempt in range(3):
        try:
            res = run_bass_kernel_spmd(nc, in_maps, list(range(NC)),
                                       trace=trace)
            break
        except Exception:
            if attempt == 2:
                raise
            import time
            time.sleep(5)
    out = np.zeros((B, S, D), dtype=np.float32)
    for core in range(NC):
        b, lc = core // 4, core % 4
        out[b, lc * NT:(lc + 1) * NT, :] = res.results[core]["out_fm"].T
    return out, res


def kernel(**inputs):
    out, _ = run(inputs)
    return out
